# revision 3
# baseline (speedup 1.0000x reference)
"""CopyMechanism (pointer-generator) kernel for 8 Trainium2 NeuronCores.

Full problem: B=16, T=128, H=512, V=32000, S=400.
  gen = sigmoid(ctx@wh + hid@ws + trg@wx + b)          [B,T,1]
  out = gen * vocab_dists; out[b,t,ids[b,t,s]] += (1-gen)*attn[b,t,s]

Sharding: data-parallel over batch; core i handles batches [2i, 2i+1]
(R=256 rows).  No cross-core communication.

v3 design (vs v1 one-hot-matmul baseline at ~373us):
 - bf16 end-to-end: vocab cast to bf16 on host (halves the dominant read),
   output written bf16 and upcast on host (halves the dominant write).
   Max |out| is O(1); bf16 rounding ~0.4% << the 2e-2 tolerance.
 - vocab/out stored p-major [128, R, 256] (v = p*250 + f, f padded to 256)
   so each partition's DMA run is 8KB contiguous per 16-row group.
 - the scatter_add is done by the GPSIMD `local_scatter` extended
   instruction: with the vocab-decomposition index p on partitions, each
   partition scatters its rows' (fi -> value) pairs independently into a
   zeroed [128, 4*256] tile (4 rows per call; indices are host-prepared
   int16 `4row*256+fi`, duplicate (pi,fi) pairs pre-summed on host,
   -1 padding ignored).  This replaces the entire one-hot A/B build +
   TensorE machinery of v1: no PSUM, no matmuls.
 - the merge out = p_gen*vocab + M is one fused DVE scalar_tensor_tensor
   per row.  p_gen is computed on-device (phase 1) and broadcast to all
   partitions via a DRAM-bounce DMA; the scatter values are pre-scaled by
   (1-p_gen) with one tiny broadcast tensor_tensor per 4-row batch.
"""

import numpy as np
from ml_dtypes import bfloat16

# problem constants (hardcoded per contract)
B, T, H, V, S = 16, 128, 512, 32000, 400
N_CORES = 8
BPC = B // N_CORES           # batches per core
R_FULL = BPC * T             # rows per core = 256
FD = 250                     # logical free width of vocab decomposition
FDP = 256                    # padded width (512B bf16 DMA runs)
NI = 16                      # index slots per (row, partition); max seen 15
RPB = 4                      # rows per local_scatter batch
NIB = RPB * NI               # idx slots per (batch, partition) = 64

_PROGRAM_CACHE = {}


def build_program(R=R_FULL, G=16, rep=1, ablate="full", merge="stt"):
    """Per-core Bass program (same program for all cores).

    R: rows per core (multiple of 128); G: rows per vocab/out DMA group;
    rep: repeat body (differential timing); ablate: "full" | "dmaonly";
    merge: "stt" (DVE scalar_tensor_tensor) | "act" (ACT base-mult via
    scale AP + DVE add).
    """
    key = (R, G, rep, ablate, merge)
    if key in _PROGRAM_CACHE:
        return _PROGRAM_CACHE[key]

    from contextlib import ExitStack

    import concourse.bass as bass
    import concourse.tile as tile
    from concourse import bacc, mybir

    f32 = mybir.dt.float32
    bf16 = mybir.dt.bfloat16
    i16 = mybir.dt.int16
    Alu = mybir.AluOpType
    Act = mybir.ActivationFunctionType
    RB = R // 128
    NG = R // G
    NB = R // RPB                # local_scatter batches
    BPG = G // RPB               # batches per DMA group
    assert R % 128 == 0 and R % G == 0 and G % RPB == 0

    nc = bacc.Bacc("TRN2", target_bir_lowering=False, debug=False)

    ctx_d = nc.dram_tensor("ctx", [R, H], bf16, kind="ExternalInput")
    hid_d = nc.dram_tensor("hid", [R, H], bf16, kind="ExternalInput")
    trg_d = nc.dram_tensor("trg", [R, H], bf16, kind="ExternalInput")
    vocab_d = nc.dram_tensor("vocab", [128, R, FDP], bf16, kind="ExternalInput")
    sidx_d = nc.dram_tensor("sidx", [128, NB, NIB], i16, kind="ExternalInput")
    sdat_d = nc.dram_tensor("sdat", [128, NB, NIB], bf16, kind="ExternalInput")
    wh_d = nc.dram_tensor("wh", [128, H], bf16, kind="ExternalInput")
    ws_d = nc.dram_tensor("ws", [128, H], bf16, kind="ExternalInput")
    wx_d = nc.dram_tensor("wx", [128, H], bf16, kind="ExternalInput")
    wxb_d = nc.dram_tensor("wxb", [128, 1], f32, kind="ExternalInput")
    out_d = nc.dram_tensor("out", [128, R, FDP], bf16, kind="ExternalOutput")

    with tile.TileContext(nc) as tc, ExitStack() as es:
        singles = es.enter_context(tc.tile_pool(name="singles", bufs=1))
        ph1 = es.enter_context(tc.tile_pool(name="ph1", bufs=2))
        vpool = es.enter_context(tc.tile_pool(name="vpool", bufs=6))
        opool = es.enter_context(tc.tile_pool(name="opool", bufs=4))
        mpool = es.enter_context(tc.tile_pool(name="mpool", bufs=6))
        dram = es.enter_context(tc.tile_pool(name="dram", bufs=1, space="DRAM"))

        sidx = singles.tile([128, NB, NIB], i16)
        nc.sync.dma_start(sidx[:], sidx_d[:])
        sdat = singles.tile([128, NB, NIB], bf16)
        nc.sync.dma_start(sdat[:], sdat_d[:])
        sdatS = singles.tile([128, NB, NIB], bf16)
        wh = singles.tile([128, H], bf16)
        nc.sync.dma_start(wh[:], wh_d[:])
        ws = singles.tile([128, H], bf16)
        nc.sync.dma_start(ws[:], ws_d[:])
        wx = singles.tile([128, H], bf16)
        nc.sync.dma_start(wx[:], wx_d[:])
        wxb = singles.tile([128, 1], f32)
        nc.sync.dma_start(wxb[:], wxb_d[:])
        pgen_all = singles.tile([128, R], f32)
        om_all = singles.tile([128, R], f32)
        pgen_dram = dram.tile([R, 1], f32)

        # --- phase 1a: p_gen per row (rows on partitions), bounce to DRAM ---
        def _phase1a():
          for blk in range(RB):
            rows = slice(blk * 128, (blk + 1) * 128)
            g0 = ph1.tile([128, 1], f32, tag="g0")
            g1 = ph1.tile([128, 1], f32, tag="g1")
            g2 = ph1.tile([128, 1], f32, tag="g2")
            gs = [g0, g1, g2]
            for i, (src_d, w) in enumerate(
                ((ctx_d, wh), (hid_d, ws), (trg_d, wx))
            ):
                x = ph1.tile([128, H], bf16, tag="x")
                nc.sync.dma_start(x[:], src_d[rows, :])
                prod = ph1.tile([128, H], f32, tag="prod")
                nc.vector.tensor_tensor(prod[:], x[:], w[:], op=Alu.mult)
                nc.vector.tensor_reduce(
                    gs[i][:], prod[:], axis=mybir.AxisListType.X, op=Alu.add
                )
            gsum = ph1.tile([128, 1], f32, tag="gsum")
            nc.vector.tensor_tensor(gsum[:], gs[0][:], gs[1][:], op=Alu.add)
            gall = ph1.tile([128, 1], f32, tag="gall")
            nc.vector.tensor_tensor(gall[:], gsum[:], gs[2][:], op=Alu.add)
            pgen_col = ph1.tile([128, 1], f32, tag="pgen")
            nc.scalar.activation(
                pgen_col[:], gall[:], Act.Sigmoid, bias=wxb[:], scale=1.0
            )
            nc.sync.dma_start(pgen_dram[rows, :], pgen_col[:])

        # --- phase 1b: broadcast p_gen; pre-scale scatter values ---
        def _phase1b():
            pg_flat = pgen_dram[:, 0]
            pg_bcast = bass.AP(
                tensor=pg_flat.tensor, offset=pg_flat.offset,
                ap=[[0, 128]] + list(pg_flat.ap),
            )
            nc.gpsimd.dma_start(pgen_all[:], pg_bcast)
            nc.vector.tensor_scalar(
                om_all[:], pgen_all[:], -1.0, 1.0, Alu.mult, Alu.add
            )
            # sdatS[p, b, j*NI+k] = sdat * (1 - pgen[4b+j])
            for b in range(NB):
                om_base = om_all[:, b * RPB]
                om_b = bass.AP(
                    tensor=om_base.tensor, offset=om_base.offset,
                    ap=[list(om_base.ap[0]), [1, RPB], [0, NI]],
                )
                s3 = sdat[:, b, 0]
                sin = bass.AP(
                    tensor=s3.tensor, offset=s3.offset,
                    ap=[list(s3.ap[0]), [NI, RPB], [1, NI]],
                )
                so = sdatS[:, b, 0]
                sout = bass.AP(
                    tensor=so.tensor, offset=so.offset,
                    ap=[list(so.ap[0]), [NI, RPB], [1, NI]],
                )
                nc.vector.tensor_tensor(sout, sin, om_b, op=Alu.mult)

        # --- phase 2: scatter via gpsimd + fused merge ---
        def _phase2():
          for g in range(NG):
            rows = slice(g * G, (g + 1) * G)
            vt = vpool.tile([128, G, FDP], bf16)
            nc.sync.dma_start(vt[:], vocab_d[:, rows, :])
            if ablate == "dmaonly":
                nc.scalar.dma_start(out_d[:, rows, :], vt[:])
                continue
            ot = opool.tile([128, G, FDP], bf16)
            for bb in range(BPG):
                b = g * BPG + bb
                M = mpool.tile([128, RPB, FDP], bf16, tag="M")
                nc.gpsimd.local_scatter(
                    M[:], sdatS[:, b, :], sidx[:, b, :],
                    channels=128, num_elems=RPB * FDP, num_idxs=NIB,
                )
                for j in range(RPB):
                    r = b * RPB + j
                    jj = bb * RPB + j
                    if merge == "act":
                        nc.scalar.activation(
                            ot[:, jj, :], vt[:, jj, :], Act.Copy,
                            scale=pgen_all[:, r:r + 1],
                        )
                        nc.vector.tensor_tensor(
                            ot[:, jj, :], ot[:, jj, :], M[:, j, :], op=Alu.add
                        )
                    else:
                        nc.vector.scalar_tensor_tensor(
                            ot[:, jj, :], vt[:, jj, :],
                            pgen_all[:, r:r + 1], M[:, j, :],
                            op0=Alu.mult, op1=Alu.add,
                        )
            nc.scalar.dma_start(out_d[:, rows, :], ot[:])

        for _ in range(rep):
            _phase1a()
            _phase1b()
            _phase2()

    nc.compile()
    _PROGRAM_CACHE[key] = nc
    return nc


def make_core_inputs(ctx, hid, trg, vocab, attn, ids, w_h, w_s, w_x_w, w_x_b,
                      R=R_FULL):
    """Host-side prep for one core: cast bf16, p-major vocab, and the
    per-partition scatter index/value arrays for local_scatter."""
    f32 = np.float32
    NB = R // RPB

    ids = np.asarray(ids).astype(np.int64)      # [R, S]
    attn = np.asarray(attn, dtype=f32)
    Sl = ids.shape[1]

    # combine duplicate (row, v) pairs, then group by (row, p)
    rr = np.repeat(np.arange(R, dtype=np.int64)[:, None], Sl, axis=1)
    key = rr.ravel() * V + ids.ravel()           # [(R*S)]
    order = np.argsort(key, kind="stable")
    ks = key[order]
    vs = attn.ravel()[order]
    uniq, start = np.unique(ks, return_index=True)
    # summed values per unique key
    sums = np.add.reduceat(vs, start)
    u_r = uniq // V
    u_v = uniq % V
    u_p = u_v // FD
    u_f = u_v % FD
    # rank within (row, p) group
    grp = u_r * 128 + u_p
    g_uniq, g_start = np.unique(grp, return_index=True)
    first = np.zeros(len(grp), dtype=np.int64)
    first[g_start] = 1
    # index of group start for each element
    gidx = np.cumsum(first) - 1
    rank = np.arange(len(grp)) - g_start[gidx]
    assert rank.max() < NI, rank.max()

    b = u_r // RPB
    j = u_r % RPB
    slot = j * NI + rank
    sidx = np.full((128, NB, NIB), -1, dtype=np.int16)
    sdat = np.zeros((128, NB, NIB), dtype=bfloat16)
    sidx[u_p, b, slot] = (j * FDP + u_f).astype(np.int16)
    sdat[u_p, b, slot] = sums.astype(bfloat16)

    def rep_w(w, n):
        return np.ascontiguousarray(
            np.broadcast_to(np.asarray(w, dtype=f32).reshape(1, n), (128, n))
        )

    bt = lambda a: np.ascontiguousarray(a).astype(bfloat16)

    vP = np.zeros((128, R, FDP), dtype=bfloat16)
    vP[:, :, :FD] = np.asarray(vocab, dtype=f32).reshape(
        R, 128, FD).transpose(1, 0, 2).astype(bfloat16)

    return {
        "ctx": bt(np.asarray(ctx, dtype=f32)),
        "hid": bt(np.asarray(hid, dtype=f32)),
        "trg": bt(np.asarray(trg, dtype=f32)),
        "vocab": vP,
        "sidx": sidx,
        "sdat": sdat,
        "wh": bt(rep_w(w_h, H)),
        "ws": bt(rep_w(w_s, H)),
        "wx": bt(rep_w(w_x_w, H)),
        "wxb": rep_w(w_x_b, 1),
    }


def make_in_maps(context_vecs, hidden, trg_embs, vocab_dists, attn_dists,
                  src_ids, w_h, w_s, w_x_w, w_x_b):
    context_vecs = np.asarray(context_vecs)
    hidden = np.asarray(hidden)
    trg_embs = np.asarray(trg_embs)
    vocab_dists = np.asarray(vocab_dists)
    attn_dists = np.asarray(attn_dists)
    src_ids = np.asarray(src_ids)

    in_maps = []
    for i in range(N_CORES):
        bs = slice(i * BPC, (i + 1) * BPC)
        in_maps.append(make_core_inputs(
            context_vecs[bs].reshape(R_FULL, H),
            hidden[bs].reshape(R_FULL, H),
            trg_embs[bs].reshape(R_FULL, H),
            vocab_dists[bs].reshape(R_FULL, V),
            attn_dists[bs].reshape(R_FULL, S),
            src_ids[bs].reshape(R_FULL, S),
            w_h, w_s, w_x_w, w_x_b,
        ))
    return in_maps


def assemble_output(raw_outs):
    """raw_outs: list of 8 arrays [128, R, 256] bf16 -> [B, T, V] f32."""
    outs = []
    for arr in raw_outs:
        a = np.asarray(arr).astype(np.float32)      # [128, R, 256]
        a = a.transpose(1, 0, 2)[:, :, :FD]          # [R, 128, 250]
        outs.append(a.reshape(BPC, T, V))
    return np.concatenate(outs, axis=0)


def kernel(context_vecs, hidden, trg_embs, vocab_dists, attn_dists,
           src_ids, pad_id, w_h, w_s, w_x_w, w_x_b):
    """Full-input entry point. Shards over 8 NeuronCores, returns [B,T,V] f32."""
    from concourse.bass_utils import run_bass_kernel_spmd

    nc = build_program()
    in_maps = make_in_maps(context_vecs, hidden, trg_embs, vocab_dists,
                            attn_dists, src_ids, w_h, w_s, w_x_w, w_x_b)
    res = run_bass_kernel_spmd(nc, in_maps, list(range(N_CORES)))
    return assemble_output([res.results[i]["out"] for i in range(N_CORES)])
